# revision 29
# baseline (speedup 1.0000x reference)
"""Trainium2 Bass kernel for nn_MultiHeadCDGCN (v4).

Math (per batch b, one batch per core across 8 cores):
  t_w  = softmax(x, axis=T);  TAtt = sum_T(x * t_w)          [N, D]
  Q    = x @ W_Q.T                                           [T, N, D]
  K    = TAtt @ W_K.T ; V = TAtt @ W_V.T                     [N, D]
  S_th = Q_th @ K_h.T / sqrt(dh)   (per t, head h)           [N, N]
  out  = (relu(S) + I) @ V = relu(S) @ V + V                 [T, N, D]

Design (vs 296us fp32 baseline):
  - fp16 everywhere on the PE (1 cyc/row, PE tiling legal): x loaded as
    fp16 via Pool SWDGE converting DMA, fp16 PE transposes, fp16 Q/K/A/V.
  - Softmax stats accumulated on the PE into PSUM via selector matmuls
    (sum over the chunk's 2 frames; PSUM accumulates across chunks),
    replacing ~95us of DVE/Pool adds.  fp16 allows the two stats to run
    in concurrent PE column bands.
  - Phase A also computes Q (resident fp16 Q.T) so phase C is a pure
    S -> relu -> A@V -> out pipeline.
  - relu(S) evacuation (16.8M PSUM elems, the hard floor: Pool cannot
    access PSUM on TRN2) is split ACT/DVE 4:4 / 5:3 alternating.
  - A@V + out path run one chunk BEHIND S/relu (software pipeline), so
    the PE always has independent work while relu drains.
  - Out: +V via scalar_tensor_tensor, PE transpose to [(t,p),(s,j)],
    one contiguous-1KB-row DMA per frame on the Sync HWDGE queue.
  - Chunk column order: col = s*128 + t*64 + p with n = s*64 + p.
"""

import sys

import numpy as np

sys.path.insert(0, "/opt/trn_rl_repo")

import concourse.bacc as bacc  # noqa: E402
import concourse.tile as tile  # noqa: E402
from concourse import mybir  # noqa: E402
from concourse.masks import make_identity  # noqa: E402
from concourse.bass_utils import run_bass_kernel_spmd  # noqa: E402

F32 = mybir.dt.float32
F16 = mybir.dt.float16
BF16 = mybir.dt.bfloat16  # bf16: 1 cyc/row on TRN2 PE (fp16 measured ~2 cyc)
AF = mybir.ActivationFunctionType

B, T, N, D, H, DH = 8, 32, 256, 256, 8, 32
P = 128
NCHUNKS = 16
CHUNK_T = 2
CHUNK_TN = CHUNK_T * N  # 512

_CACHE: dict = {}


def _build_program():
    nc = bacc.Bacc()

    x_d = nc.dram_tensor("x", [T, N, D], F32, kind="ExternalInput")
    wqt_d = nc.dram_tensor("wqt", [D, D], F16, kind="ExternalInput")
    wkt_d = nc.dram_tensor("wkt", [D, D], F16, kind="ExternalInput")
    wvt_d = nc.dram_tensor("wvt", [D, D], F16, kind="ExternalInput")
    sel_d = nc.dram_tensor("sel", [P, 64], F16, kind="ExternalInput")
    out_d = nc.dram_tensor("out", [T, N, D], F32, kind="ExternalOutput")

    with tile.TileContext(nc) as tc:
        with (
            tc.tile_pool(name="consts", bufs=1) as consts,
            tc.tile_pool(name="xa", bufs=6) as xa_pool,
            tc.tile_pool(name="ew", bufs=3) as e_pool,
            tc.tile_pool(name="at", bufs=16) as a_pool,
            tc.tile_pool(name="ot", bufs=4) as o_pool,
            tc.tile_pool(name="misc", bufs=1) as misc,
            tc.tile_pool(name="ps", bufs=1, space="PSUM") as ps,
        ):
            eye = consts.tile([P, P], F32)
            make_identity(nc, eye)
            eye16 = consts.tile([P, P], F16)
            nc.vector.tensor_copy(eye16, eye)
            eye_b = consts.tile([P, P], BF16)
            nc.vector.tensor_copy(eye_b, eye)

            sel_sb = consts.tile([P, 64], F16)
            nc.sync.dma_start(out=sel_sb, in_=sel_d[:, :])

            wqt_sb = consts.tile([P, 2, D], F16)
            wkt_sb = consts.tile([P, 2, D], F16)
            wvt_sb = consts.tile([P, 2, D], F16)
            for w_sb, w_d in ((wqt_sb, wqt_d), (wkt_sb, wkt_d), (wvt_sb, wvt_d)):
                for kc in range(2):
                    nc.sync.dma_start(
                        out=w_sb[:, kc, :],
                        in_=w_d[kc * P : (kc + 1) * P, :],
                    )

            # Residents (fp16): x.T and Q.T, [128, 2, 8192] each (4 MB).
            xt_res = consts.tile([P, 2, T * N], F16)
            qt_res = consts.tile([P, 2, T * N], BF16)

            # Softmax stats in PSUM (accumulate across chunks):
            # rows 0:64 sum_e, rows 64:128 sum_xe (concurrent col bands).
            pstat = ps.tile([P, 1024], F32, tag="b2", bufs=3, name="pstat")

            # ---------------- Phase A: stream x; stats; x.T; Q.T
            # PE queue is in-order: transposes(c) depend only on the
            # (prefetched) x chunk, while stats(c)/Q(c) wait on ACT/DVE
            # products.  Emitting stats/Q one chunk late keeps the PE fed.
            xa_t = {}
            e_t = {}

            def a_load(c):
                t0 = c * CHUNK_T
                xa = xa_pool.tile([P, 4, D], F16, name="xa")
                for ti in range(CHUNK_T):
                    # Pool SWDGE converts fp32 -> fp16 during the load.
                    nc.gpsimd.dma_start(
                        out=xa[ti * 64 : (ti + 1) * 64],
                        in_=x_d[t0 + ti].rearrange("(s p) d -> p s d", p=64),
                    )
                xa_t[c] = xa

            def a_transposes(c):
                xa = xa_t[c]
                for dc in range(2):
                    pt = ps.tile([P, CHUNK_TN], F16, tag="b1", bufs=2, name="pt")
                    for s in range(4):
                        for ph in range(2):
                            # Two concurrent 64-col-band transposes.
                            nc.tensor.transpose(
                                pt[ph * 64 : (ph + 1) * 64, s * P : (s + 1) * P],
                                xa[
                                    :,
                                    s,
                                    dc * P + ph * 64 : dc * P + (ph + 1) * 64,
                                ],
                                eye16,
                                tile_position=(0, ph * 64),
                            )
                    nc.vector.tensor_copy(
                        xt_res[:, dc, c * CHUNK_TN : (c + 1) * CHUNK_TN], pt
                    )

            def a_exp(c):
                xa = xa_t[c]
                e2 = e_pool.tile([P, 4 * D], F16, tag="e2", name="e2")
                nc.scalar.activation(e2, xa.rearrange("p s d -> p (s d)"), AF.Exp)
                xe2 = e_pool.tile([P, 4 * D], F16, tag="xe2", name="xe2")
                nc.vector.tensor_mul(xe2, xa.rearrange("p s d -> p (s d)"), e2)
                e_t[c] = (e2, xe2)

            def a_stats(c):
                e2, xe2 = e_t.pop(c)
                for half in range(2):
                    nc.tensor.matmul(
                        pstat[0:64, half * 512 : (half + 1) * 512],
                        sel_sb,
                        e2[:, half * 512 : (half + 1) * 512],
                        start=(c == 0),
                        stop=(c == NCHUNKS - 1),
                        tile_position=(0, 0),
                        skip_group_check=True,
                    )
                    nc.tensor.matmul(
                        pstat[64:128, half * 512 : (half + 1) * 512],
                        sel_sb,
                        xe2[:, half * 512 : (half + 1) * 512],
                        start=(c == 0),
                        stop=(c == NCHUNKS - 1),
                        tile_position=(0, 64),
                        skip_group_check=True,
                    )

            def a_q(c):
                xa_t.pop(c, None)
                for jc in range(2):
                    pq = ps.tile([P, CHUNK_TN], F32, tag="b1", bufs=2, name="pq")
                    for kc in range(2):
                        nc.tensor.matmul(
                            pq,
                            wqt_sb[:, kc, jc * P : (jc + 1) * P],
                            xt_res[:, kc, c * CHUNK_TN : (c + 1) * CHUNK_TN],
                            start=(kc == 0),
                            stop=(kc == 1),
                        )
                    dst = qt_res[:, jc, c * CHUNK_TN : (c + 1) * CHUNK_TN]
                    if jc == 0:
                        nc.scalar.activation(dst, pq, AF.Copy)
                    else:
                        nc.vector.tensor_copy(dst, pq)

            a_load(0)
            a_load(1)
            for c in range(NCHUNKS):
                if c + 2 < NCHUNKS:
                    a_load(c + 2)
                a_exp(c)
                a_transposes(c)
                if c >= 1:
                    a_stats(c - 1)
                    a_q(c - 1)
            a_stats(NCHUNKS - 1)
            a_q(NCHUNKS - 1)

            # ---------------- Phase B: TAtt, K, V, vt2
            se_sb = misc.tile([64, 1024], F32)
            nc.scalar.activation(se_sb, pstat[0:64, :], AF.Copy)
            sxe_sb = misc.tile([64, 1024], F32)
            nc.vector.tensor_copy(sxe_sb, pstat[64:128, :])
            rec = misc.tile([64, 1024], F32)
            nc.vector.reciprocal_approx_fast(out=rec, in_=se_sb)
            tatt_nat = misc.tile([64, 1024], F32)  # [p64, (s4, d256)]
            nc.vector.tensor_mul(tatt_nat, sxe_sb, rec)

            # TAtt.T [d, n] fp16 (n = s*64+p).
            tatt_t = consts.tile([P, 2, N], F16)
            for dc in range(2):
                ptb = ps.tile([P, N], F32, tag="b1", bufs=2, name="ptb")
                for s in range(4):
                    nc.tensor.transpose(
                        ptb[:, s * 64 : (s + 1) * 64],
                        tatt_nat[:, s * 256 + dc * P : s * 256 + (dc + 1) * P],
                        eye[0:64, 0:64],
                    )
                nc.vector.tensor_copy(tatt_t[:, dc, :], ptb)

            kt_sb = consts.tile([P, 2, N], BF16)  # K.T [j, m], pre-scaled
            for jc in range(2):
                pk = ps.tile([P, N], F32, tag="b1", bufs=2, name="pk")
                for kc in range(2):
                    nc.tensor.matmul(
                        pk,
                        wkt_sb[:, kc, jc * P : (jc + 1) * P],
                        tatt_t[:, kc, :],
                        start=(kc == 0),
                        stop=(kc == 1),
                    )
                nc.scalar.activation(kt_sb[:, jc, :], pk, AF.Copy)

            v_sb = consts.tile([P, 2, D], BF16)  # V [m, j]
            for mc in range(2):
                pv = ps.tile([P, D], F32, tag="b1", bufs=2, name="pv")
                for kc in range(2):
                    nc.tensor.matmul(
                        pv,
                        tatt_t[:, kc, mc * P : (mc + 1) * P],
                        wvt_sb[:, kc, :],
                        start=(kc == 0),
                        stop=(kc == 1),
                    )
                nc.vector.tensor_copy(v_sb[:, mc, :], pv)

            vt_sb = misc.tile([P, 2, N], F32)  # V.T [j, n]
            for jc in range(2):
                pt2 = ps.tile([P, N], BF16, tag="b1", bufs=2, name="pt2")
                for mc in range(2):
                    nc.tensor.transpose(
                        pt2[:, mc * P : (mc + 1) * P],
                        v_sb[:, mc, jc * P : (jc + 1) * P],
                        eye_b,
                    )
                nc.vector.tensor_copy(vt_sb[:, jc, :], pt2)

            # vt2: V.T in chunk column order (col = s*128 + t*64 + p).
            vt2 = consts.tile([P, 2, CHUNK_TN], F32)
            for hg in range(2):
                for s in range(4):
                    for ti in range(CHUNK_T):
                        nc.gpsimd.tensor_copy(
                            vt2[:, hg, s * P + ti * 64 : s * P + ti * 64 + 64],
                            vt_sb[:, hg, s * 64 : (s + 1) * 64],
                        )

            # ---------------- Phase C: S -> relu; A@V/out one chunk behind
            a_store = {}

            def s_phase_hg(c, hg):
                nrelu = 4 * hg
                if True:
                    for mc in range(2):
                        for rp in range(2):
                            ps2 = ps.tile(
                                [P, 2 * CHUNK_TN],
                                F32,
                                tag="b2",
                                bufs=3,
                                name=f"ps{hg}{mc}{rp}",
                            )
                            for rh in range(2):
                                r = rp * 2 + rh
                                nc.tensor.matmul(
                                    ps2[:, rh * CHUNK_TN : (rh + 1) * CHUNK_TN],
                                    kt_sb[
                                        r * 32 : (r + 1) * 32,
                                        hg,
                                        mc * P : (mc + 1) * P,
                                    ],
                                    qt_res[
                                        r * 32 : (r + 1) * 32,
                                        hg,
                                        c * CHUNK_TN : (c + 1) * CHUNK_TN,
                                    ],
                                    start=True,
                                    stop=True,
                                    tile_position=(r * 32, 0),
                                )
                            a2 = a_pool.tile(
                                [P, 2 * CHUNK_TN],
                                BF16,
                                tag="at",
                                name=f"a{hg}{mc}{rp}",
                            )
                            n_act = 4 if c % 2 == 0 else 5
                            if nrelu < n_act:
                                nc.scalar.activation(a2, ps2, AF.Relu)
                            else:
                                nc.vector.tensor_scalar_max(a2, ps2, 0.0)
                            nrelu += 1
                            for rh in range(2):
                                a_store[(c, hg, rp * 2 + rh, mc)] = a2[
                                    :, rh * CHUNK_TN : (rh + 1) * CHUNK_TN
                                ]

            o_sb_store = {}

            def av_phase_hg(c, hg):
                po = ps.tile(
                    [P, CHUNK_TN], F32, tag="b1", bufs=2, name=f"po{hg}"
                )
                for mc in range(2):
                    for r in range(4):
                        h = hg * 4 + r
                        nc.tensor.matmul(
                            po[r * 32 : (r + 1) * 32, :],
                            v_sb[:, mc, h * 32 : (h + 1) * 32],
                            a_store.pop((c, hg, r, mc)),
                            start=(mc == 0),
                            stop=(mc == 1),
                            tile_position=(0, r * 32),
                            skip_group_check=True,
                        )
                o_sb = o_pool.tile(
                    [P, CHUNK_TN], F32, tag=f"ob{hg}", name="o_sb"
                )
                nc.vector.scalar_tensor_tensor(
                    out=o_sb,
                    in0=po,
                    scalar=1.0,
                    in1=vt2[:, hg, :],
                    op0=mybir.AluOpType.mult,
                    op1=mybir.AluOpType.add,
                )
                o_sb_store[(c, hg)] = o_sb

            def out_tail(c):
                t0 = c * CHUNK_T
                o_sbs = [o_sb_store.pop((c, 0)), o_sb_store.pop((c, 1))]
                # PE transpose -> [(t, p), (s, hg, j)]; contiguous out rows.
                po2 = ps.tile(
                    [P, 2 * CHUNK_TN], F32, tag="b2", bufs=3, name="po2"
                )
                for hg in range(2):
                    for s in range(4):
                        nc.tensor.transpose(
                            po2[:, s * 256 + hg * P : s * 256 + (hg + 1) * P],
                            o_sbs[hg][:, s * P : (s + 1) * P],
                            eye,
                        )
                o2 = o_pool.tile([P, 2 * CHUNK_TN], F32, tag="o2", name="o2")
                nc.scalar.activation(o2, po2, AF.Copy)
                for ti in range(CHUNK_T):
                    nc.sync.dma_start(
                        out=out_d[t0 + ti].rearrange("(s p) d -> p s d", p=64),
                        in_=o2[ti * 64 : (ti + 1) * 64].rearrange(
                            "p (s d) -> p s d", s=4
                        ),
                    )

            for c in range(NCHUNKS):
                s_phase_hg(c, 0)
                s_phase_hg(c, 1)
                if c >= 1:
                    av_phase_hg(c - 1, 0)
                    av_phase_hg(c - 1, 1)
                    out_tail(c - 1)
            av_phase_hg(NCHUNKS - 1, 0)
            av_phase_hg(NCHUNKS - 1, 1)
            out_tail(NCHUNKS - 1)

    nc.finalize()
    return nc


def _in_maps(inputs) -> list:
    x = np.ascontiguousarray(np.asarray(inputs["x"], dtype=np.float32))
    w_q = np.asarray(inputs["W_Q"], dtype=np.float32)
    w_k = np.asarray(inputs["W_K"], dtype=np.float32)
    w_v = np.asarray(inputs["W_V"], dtype=np.float32)

    wqt = np.ascontiguousarray(w_q.T).astype(np.float16)
    wkt = (np.ascontiguousarray(w_k.T) * np.float32(1.0 / np.sqrt(DH))).astype(
        np.float16
    )
    wvt = np.ascontiguousarray(w_v.T).astype(np.float16)
    sel = np.zeros((P, 64), dtype=np.float16)
    for t in range(CHUNK_T):
        sel[t * 64 : (t + 1) * 64] = np.eye(64, dtype=np.float16)

    return [
        {
            "x": np.ascontiguousarray(x[b]),
            "wqt": wqt,
            "wkt": wkt,
            "wvt": wvt,
            "sel": sel,
        }
        for b in range(B)
    ]


def kernel(**inputs) -> np.ndarray:
    if "nc" not in _CACHE:
        _CACHE["nc"] = _build_program()
    nc = _CACHE["nc"]
    in_maps = _in_maps(inputs)
    res = run_bass_kernel_spmd(nc, in_maps, core_ids=list(range(B)))
    out = np.stack([res.results[b]["out"] for b in range(B)], axis=0)
    return out.reshape(B, T, N, D)


# revision 30
# speedup vs baseline: 1.1243x; 1.1243x over previous
"""Trainium2 Bass kernel for nn_MultiHeadCDGCN (v4).

Math (per batch b, one batch per core across 8 cores):
  t_w  = softmax(x, axis=T);  TAtt = sum_T(x * t_w)          [N, D]
  Q    = x @ W_Q.T                                           [T, N, D]
  K    = TAtt @ W_K.T ; V = TAtt @ W_V.T                     [N, D]
  S_th = Q_th @ K_h.T / sqrt(dh)   (per t, head h)           [N, N]
  out  = (relu(S) + I) @ V = relu(S) @ V + V                 [T, N, D]

Design (vs 296us fp32 baseline):
  - fp16 everywhere on the PE (1 cyc/row, PE tiling legal): x loaded as
    fp16 via Pool SWDGE converting DMA, fp16 PE transposes, fp16 Q/K/A/V.
  - Softmax stats accumulated on the PE into PSUM via selector matmuls
    (sum over the chunk's 2 frames; PSUM accumulates across chunks),
    replacing ~95us of DVE/Pool adds.  fp16 allows the two stats to run
    in concurrent PE column bands.
  - Phase A also computes Q (resident fp16 Q.T) so phase C is a pure
    S -> relu -> A@V -> out pipeline.
  - relu(S) evacuation (16.8M PSUM elems, the hard floor: Pool cannot
    access PSUM on TRN2) is split ACT/DVE 4:4 / 5:3 alternating.
  - A@V + out path run one chunk BEHIND S/relu (software pipeline), so
    the PE always has independent work while relu drains.
  - Out: +V via scalar_tensor_tensor, PE transpose to [(t,p),(s,j)],
    one contiguous-1KB-row DMA per frame on the Sync HWDGE queue.
  - Chunk column order: col = s*128 + t*64 + p with n = s*64 + p.
"""

import sys

import numpy as np

sys.path.insert(0, "/opt/trn_rl_repo")

import concourse.bacc as bacc  # noqa: E402
import concourse.tile as tile  # noqa: E402
from concourse import mybir  # noqa: E402
from concourse.masks import make_identity  # noqa: E402
from concourse.bass_utils import run_bass_kernel_spmd  # noqa: E402

F32 = mybir.dt.float32
F16 = mybir.dt.float16
BF16 = mybir.dt.bfloat16  # bf16: 1 cyc/row on TRN2 PE (fp16 measured ~2 cyc)
AF = mybir.ActivationFunctionType

B, T, N, D, H, DH = 8, 32, 256, 256, 8, 32
P = 128
NCHUNKS = 16
CHUNK_T = 2
CHUNK_TN = CHUNK_T * N  # 512

_CACHE: dict = {}


def _build_program():
    nc = bacc.Bacc()

    x_d = nc.dram_tensor("x", [T, N, D], F32, kind="ExternalInput")
    wqt_d = nc.dram_tensor("wqt", [D, D], F16, kind="ExternalInput")
    wkt_d = nc.dram_tensor("wkt", [D, D], F16, kind="ExternalInput")
    wvt_d = nc.dram_tensor("wvt", [D, D], F16, kind="ExternalInput")
    sel_d = nc.dram_tensor("sel", [P, 64], F16, kind="ExternalInput")
    out_d = nc.dram_tensor("out", [T, N, D], F32, kind="ExternalOutput")

    with tile.TileContext(nc) as tc:
        with (
            tc.tile_pool(name="consts", bufs=1) as consts,
            tc.tile_pool(name="xa", bufs=6) as xa_pool,
            tc.tile_pool(name="ew", bufs=3) as e_pool,
            tc.tile_pool(name="at", bufs=16) as a_pool,
            tc.tile_pool(name="ot", bufs=4) as o_pool,
            tc.tile_pool(name="misc", bufs=1) as misc,
            tc.tile_pool(name="ps", bufs=1, space="PSUM") as ps,
        ):
            eye = consts.tile([P, P], F32)
            make_identity(nc, eye)
            eye16 = consts.tile([P, P], F16)
            nc.vector.tensor_copy(eye16, eye)
            eye_b = consts.tile([P, P], BF16)
            nc.vector.tensor_copy(eye_b, eye)

            sel_sb = consts.tile([P, 64], F16)
            nc.sync.dma_start(out=sel_sb, in_=sel_d[:, :])

            wqt_sb = consts.tile([P, 2, D], F16)
            wkt_sb = consts.tile([P, 2, D], F16)
            wvt_sb = consts.tile([P, 2, D], F16)
            for w_sb, w_d in ((wqt_sb, wqt_d), (wkt_sb, wkt_d), (wvt_sb, wvt_d)):
                for kc in range(2):
                    nc.sync.dma_start(
                        out=w_sb[:, kc, :],
                        in_=w_d[kc * P : (kc + 1) * P, :],
                    )

            # Residents (fp16): x.T and Q.T, [128, 2, 8192] each (4 MB).
            xt_res = consts.tile([P, 2, T * N], F16)
            qt_res = consts.tile([P, 2, T * N], BF16)

            # Softmax stats in PSUM (accumulate across chunks):
            # rows 0:64 sum_e, rows 64:128 sum_xe (concurrent col bands).
            pstat = ps.tile([P, 1024], F32, tag="b2", bufs=3, name="pstat")

            # ---------------- Phase A: stream x; stats; x.T; Q.T
            # PE queue is in-order: transposes(c) depend only on the
            # (prefetched) x chunk, while stats(c)/Q(c) wait on ACT/DVE
            # products.  Emitting stats/Q one chunk late keeps the PE fed.
            xa_t = {}
            e_t = {}

            def a_load(c):
                t0 = c * CHUNK_T
                xa = xa_pool.tile([P, 4, D], F16, name="xa")
                for ti in range(CHUNK_T):
                    # Pool SWDGE converts fp32 -> fp16 during the load.
                    nc.gpsimd.dma_start(
                        out=xa[ti * 64 : (ti + 1) * 64],
                        in_=x_d[t0 + ti].rearrange("(s p) d -> p s d", p=64),
                    )
                xa_t[c] = xa

            def a_transposes(c):
                xa = xa_t[c]
                for dc in range(2):
                    pt = ps.tile([P, CHUNK_TN], F16, tag="b1", bufs=2, name="pt")
                    for s in range(4):
                        nc.tensor.transpose(
                            pt[:, s * P : (s + 1) * P],
                            xa[:, s, dc * P : (dc + 1) * P],
                            eye16,
                        )
                    nc.vector.tensor_copy(
                        xt_res[:, dc, c * CHUNK_TN : (c + 1) * CHUNK_TN], pt
                    )

            def a_exp(c):
                xa = xa_t[c]
                e2 = e_pool.tile([P, 4 * D], F16, tag="e2", name="e2")
                nc.scalar.activation(e2, xa.rearrange("p s d -> p (s d)"), AF.Exp)
                xe2 = e_pool.tile([P, 4 * D], F16, tag="xe2", name="xe2")
                nc.vector.tensor_mul(xe2, xa.rearrange("p s d -> p (s d)"), e2)
                e_t[c] = (e2, xe2)

            def a_stats(c):
                e2, xe2 = e_t.pop(c)
                for half in range(2):
                    nc.tensor.matmul(
                        pstat[0:64, half * 512 : (half + 1) * 512],
                        sel_sb,
                        e2[:, half * 512 : (half + 1) * 512],
                        start=(c == 0),
                        stop=(c == NCHUNKS - 1),
                        tile_position=(0, 0),
                        skip_group_check=True,
                    )
                    nc.tensor.matmul(
                        pstat[64:128, half * 512 : (half + 1) * 512],
                        sel_sb,
                        xe2[:, half * 512 : (half + 1) * 512],
                        start=(c == 0),
                        stop=(c == NCHUNKS - 1),
                        tile_position=(0, 64),
                        skip_group_check=True,
                    )

            def a_q(c):
                xa_t.pop(c, None)
                for jc in range(2):
                    pq = ps.tile([P, CHUNK_TN], F32, tag="b1", bufs=2, name="pq")
                    for kc in range(2):
                        nc.tensor.matmul(
                            pq,
                            wqt_sb[:, kc, jc * P : (jc + 1) * P],
                            xt_res[:, kc, c * CHUNK_TN : (c + 1) * CHUNK_TN],
                            start=(kc == 0),
                            stop=(kc == 1),
                        )
                    dst = qt_res[:, jc, c * CHUNK_TN : (c + 1) * CHUNK_TN]
                    if jc == 0:
                        nc.scalar.activation(dst, pq, AF.Copy)
                    else:
                        nc.vector.tensor_copy(dst, pq)

            a_load(0)
            a_load(1)
            for c in range(NCHUNKS):
                if c + 2 < NCHUNKS:
                    a_load(c + 2)
                a_exp(c)
                a_transposes(c)
                if c >= 1:
                    a_stats(c - 1)
                    a_q(c - 1)
            a_stats(NCHUNKS - 1)
            a_q(NCHUNKS - 1)

            # ---------------- Phase B: TAtt, K, V, vt2
            se_sb = misc.tile([64, 1024], F32)
            nc.scalar.activation(se_sb, pstat[0:64, :], AF.Copy)
            sxe_sb = misc.tile([64, 1024], F32)
            nc.vector.tensor_copy(sxe_sb, pstat[64:128, :])
            rec = misc.tile([64, 1024], F32)
            nc.vector.reciprocal_approx_fast(out=rec, in_=se_sb)
            tatt_nat = misc.tile([64, 1024], F32)  # [p64, (s4, d256)]
            nc.vector.tensor_mul(tatt_nat, sxe_sb, rec)

            # TAtt.T [d, n] fp16 (n = s*64+p).
            tatt_t = consts.tile([P, 2, N], F16)
            for dc in range(2):
                ptb = ps.tile([P, N], F32, tag="b1", bufs=2, name="ptb")
                for s in range(4):
                    nc.tensor.transpose(
                        ptb[:, s * 64 : (s + 1) * 64],
                        tatt_nat[:, s * 256 + dc * P : s * 256 + (dc + 1) * P],
                        eye[0:64, 0:64],
                    )
                nc.vector.tensor_copy(tatt_t[:, dc, :], ptb)

            kt_sb = consts.tile([P, 2, N], BF16)  # K.T [j, m], pre-scaled
            for jc in range(2):
                pk = ps.tile([P, N], F32, tag="b1", bufs=2, name="pk")
                for kc in range(2):
                    nc.tensor.matmul(
                        pk,
                        wkt_sb[:, kc, jc * P : (jc + 1) * P],
                        tatt_t[:, kc, :],
                        start=(kc == 0),
                        stop=(kc == 1),
                    )
                nc.scalar.activation(kt_sb[:, jc, :], pk, AF.Copy)

            v_sb = consts.tile([P, 2, D], BF16)  # V [m, j]
            for mc in range(2):
                pv = ps.tile([P, D], F32, tag="b1", bufs=2, name="pv")
                for kc in range(2):
                    nc.tensor.matmul(
                        pv,
                        tatt_t[:, kc, mc * P : (mc + 1) * P],
                        wvt_sb[:, kc, :],
                        start=(kc == 0),
                        stop=(kc == 1),
                    )
                nc.vector.tensor_copy(v_sb[:, mc, :], pv)

            vt_sb = misc.tile([P, 2, N], F32)  # V.T [j, n]
            for jc in range(2):
                pt2 = ps.tile([P, N], BF16, tag="b1", bufs=2, name="pt2")
                for mc in range(2):
                    nc.tensor.transpose(
                        pt2[:, mc * P : (mc + 1) * P],
                        v_sb[:, mc, jc * P : (jc + 1) * P],
                        eye_b,
                    )
                nc.vector.tensor_copy(vt_sb[:, jc, :], pt2)

            # vt2: V.T in chunk column order (col = s*128 + t*64 + p).
            vt2 = consts.tile([P, 2, CHUNK_TN], F32)
            for hg in range(2):
                for s in range(4):
                    for ti in range(CHUNK_T):
                        nc.gpsimd.tensor_copy(
                            vt2[:, hg, s * P + ti * 64 : s * P + ti * 64 + 64],
                            vt_sb[:, hg, s * 64 : (s + 1) * 64],
                        )

            # ---------------- Phase C: S -> relu; A@V/out one chunk behind
            a_store = {}

            def s_phase_hg(c, hg):
                nrelu = 4 * hg
                if True:
                    for mc in range(2):
                        for rp in range(2):
                            ps2 = ps.tile(
                                [P, 2 * CHUNK_TN],
                                F32,
                                tag="b2",
                                bufs=3,
                                name=f"ps{hg}{mc}{rp}",
                            )
                            for rh in range(2):
                                r = rp * 2 + rh
                                nc.tensor.matmul(
                                    ps2[:, rh * CHUNK_TN : (rh + 1) * CHUNK_TN],
                                    kt_sb[
                                        r * 32 : (r + 1) * 32,
                                        hg,
                                        mc * P : (mc + 1) * P,
                                    ],
                                    qt_res[
                                        r * 32 : (r + 1) * 32,
                                        hg,
                                        c * CHUNK_TN : (c + 1) * CHUNK_TN,
                                    ],
                                    start=True,
                                    stop=True,
                                    tile_position=(r * 32, 0),
                                )
                            a2 = a_pool.tile(
                                [P, 2 * CHUNK_TN],
                                BF16,
                                tag="at",
                                name=f"a{hg}{mc}{rp}",
                            )
                            n_act = 4 if c % 2 == 0 else 5
                            if nrelu < n_act:
                                nc.scalar.activation(a2, ps2, AF.Relu)
                            else:
                                nc.vector.tensor_scalar_max(a2, ps2, 0.0)
                            nrelu += 1
                            for rh in range(2):
                                a_store[(c, hg, rp * 2 + rh, mc)] = a2[
                                    :, rh * CHUNK_TN : (rh + 1) * CHUNK_TN
                                ]

            o_sb_store = {}

            def av_phase_hg(c, hg):
                po = ps.tile(
                    [P, CHUNK_TN], F32, tag="b1", bufs=2, name=f"po{hg}"
                )
                for mc in range(2):
                    for r in range(4):
                        h = hg * 4 + r
                        nc.tensor.matmul(
                            po[r * 32 : (r + 1) * 32, :],
                            v_sb[:, mc, h * 32 : (h + 1) * 32],
                            a_store.pop((c, hg, r, mc)),
                            start=(mc == 0),
                            stop=(mc == 1),
                            tile_position=(0, r * 32),
                            skip_group_check=True,
                        )
                o_sb = o_pool.tile(
                    [P, CHUNK_TN], F32, tag=f"ob{hg}", name="o_sb"
                )
                nc.vector.scalar_tensor_tensor(
                    out=o_sb,
                    in0=po,
                    scalar=1.0,
                    in1=vt2[:, hg, :],
                    op0=mybir.AluOpType.mult,
                    op1=mybir.AluOpType.add,
                )
                o_sb_store[(c, hg)] = o_sb

            def out_tail(c):
                t0 = c * CHUNK_T
                o_sbs = [o_sb_store.pop((c, 0)), o_sb_store.pop((c, 1))]
                # PE transpose -> [(t, p), (s, hg, j)]; contiguous out rows.
                po2 = ps.tile(
                    [P, 2 * CHUNK_TN], F32, tag="b2", bufs=3, name="po2"
                )
                for hg in range(2):
                    for s in range(4):
                        nc.tensor.transpose(
                            po2[:, s * 256 + hg * P : s * 256 + (hg + 1) * P],
                            o_sbs[hg][:, s * P : (s + 1) * P],
                            eye,
                        )
                o2 = o_pool.tile([P, 2 * CHUNK_TN], F32, tag="o2", name="o2")
                nc.scalar.activation(o2, po2, AF.Copy)
                for ti in range(CHUNK_T):
                    nc.sync.dma_start(
                        out=out_d[t0 + ti].rearrange("(s p) d -> p s d", p=64),
                        in_=o2[ti * 64 : (ti + 1) * 64].rearrange(
                            "p (s d) -> p s d", s=4
                        ),
                    )

            for c in range(NCHUNKS):
                s_phase_hg(c, 0)
                s_phase_hg(c, 1)
                if c >= 1:
                    av_phase_hg(c - 1, 0)
                    av_phase_hg(c - 1, 1)
                    out_tail(c - 1)
            av_phase_hg(NCHUNKS - 1, 0)
            av_phase_hg(NCHUNKS - 1, 1)
            out_tail(NCHUNKS - 1)

    nc.finalize()
    return nc


def _in_maps(inputs) -> list:
    x = np.ascontiguousarray(np.asarray(inputs["x"], dtype=np.float32))
    w_q = np.asarray(inputs["W_Q"], dtype=np.float32)
    w_k = np.asarray(inputs["W_K"], dtype=np.float32)
    w_v = np.asarray(inputs["W_V"], dtype=np.float32)

    wqt = np.ascontiguousarray(w_q.T).astype(np.float16)
    wkt = (np.ascontiguousarray(w_k.T) * np.float32(1.0 / np.sqrt(DH))).astype(
        np.float16
    )
    wvt = np.ascontiguousarray(w_v.T).astype(np.float16)
    sel = np.zeros((P, 64), dtype=np.float16)
    for t in range(CHUNK_T):
        sel[t * 64 : (t + 1) * 64] = np.eye(64, dtype=np.float16)

    return [
        {
            "x": np.ascontiguousarray(x[b]),
            "wqt": wqt,
            "wkt": wkt,
            "wvt": wvt,
            "sel": sel,
        }
        for b in range(B)
    ]


def kernel(**inputs) -> np.ndarray:
    if "nc" not in _CACHE:
        _CACHE["nc"] = _build_program()
    nc = _CACHE["nc"]
    in_maps = _in_maps(inputs)
    res = run_bass_kernel_spmd(nc, in_maps, core_ids=list(range(B)))
    out = np.stack([res.results[b]["out"] for b in range(B)], axis=0)
    return out.reshape(B, T, N, D)


# revision 31
# speedup vs baseline: 1.1507x; 1.0235x over previous
"""Trainium2 Bass kernel for nn_MultiHeadCDGCN (v4).

Math (per batch b, one batch per core across 8 cores):
  t_w  = softmax(x, axis=T);  TAtt = sum_T(x * t_w)          [N, D]
  Q    = x @ W_Q.T                                           [T, N, D]
  K    = TAtt @ W_K.T ; V = TAtt @ W_V.T                     [N, D]
  S_th = Q_th @ K_h.T / sqrt(dh)   (per t, head h)           [N, N]
  out  = (relu(S) + I) @ V = relu(S) @ V + V                 [T, N, D]

Design (vs 296us fp32 baseline):
  - fp16 everywhere on the PE (1 cyc/row, PE tiling legal): x loaded as
    fp16 via Pool SWDGE converting DMA, fp16 PE transposes, fp16 Q/K/A/V.
  - Softmax stats accumulated on the PE into PSUM via selector matmuls
    (sum over the chunk's 2 frames; PSUM accumulates across chunks),
    replacing ~95us of DVE/Pool adds.  fp16 allows the two stats to run
    in concurrent PE column bands.
  - Phase A also computes Q (resident fp16 Q.T) so phase C is a pure
    S -> relu -> A@V -> out pipeline.
  - relu(S) evacuation (16.8M PSUM elems, the hard floor: Pool cannot
    access PSUM on TRN2) is split ACT/DVE 4:4 / 5:3 alternating.
  - A@V + out path run one chunk BEHIND S/relu (software pipeline), so
    the PE always has independent work while relu drains.
  - Out: +V via scalar_tensor_tensor, PE transpose to [(t,p),(s,j)],
    one contiguous-1KB-row DMA per frame on the Sync HWDGE queue.
  - Chunk column order: col = s*128 + t*64 + p with n = s*64 + p.
"""

import sys

import numpy as np

sys.path.insert(0, "/opt/trn_rl_repo")

import concourse.bacc as bacc  # noqa: E402
import concourse.tile as tile  # noqa: E402
from concourse import mybir  # noqa: E402
from concourse.masks import make_identity  # noqa: E402
from concourse.bass_utils import run_bass_kernel_spmd  # noqa: E402

F32 = mybir.dt.float32
F16 = mybir.dt.float16
BF16 = mybir.dt.bfloat16  # bf16: 1 cyc/row on TRN2 PE (fp16 measured ~2 cyc)
AF = mybir.ActivationFunctionType

B, T, N, D, H, DH = 8, 32, 256, 256, 8, 32
P = 128
NCHUNKS = 16
CHUNK_T = 2
CHUNK_TN = CHUNK_T * N  # 512

_CACHE: dict = {}


def _build_program():
    nc = bacc.Bacc()

    x_d = nc.dram_tensor("x", [T, N, D], F32, kind="ExternalInput")
    wqt_d = nc.dram_tensor("wqt", [D, D], F16, kind="ExternalInput")
    wkt_d = nc.dram_tensor("wkt", [D, D], F16, kind="ExternalInput")
    wvt_d = nc.dram_tensor("wvt", [D, D], F16, kind="ExternalInput")
    sel_d = nc.dram_tensor("sel", [P, 64], F16, kind="ExternalInput")
    out_d = nc.dram_tensor("out", [T, N, D], F32, kind="ExternalOutput")

    with tile.TileContext(nc) as tc:
        with (
            tc.tile_pool(name="consts", bufs=1) as consts,
            tc.tile_pool(name="xa", bufs=6) as xa_pool,
            tc.tile_pool(name="ew", bufs=3) as e_pool,
            tc.tile_pool(name="at", bufs=16) as a_pool,
            tc.tile_pool(name="ot", bufs=4) as o_pool,
            tc.tile_pool(name="misc", bufs=1) as misc,
            tc.tile_pool(name="ps", bufs=1, space="PSUM") as ps,
        ):
            eye = consts.tile([P, P], F32)
            make_identity(nc, eye)
            eye16 = consts.tile([P, P], F16)
            nc.vector.tensor_copy(eye16, eye)
            eye_b = consts.tile([P, P], BF16)
            nc.vector.tensor_copy(eye_b, eye)

            sel_sb = consts.tile([P, 64], F16)
            nc.sync.dma_start(out=sel_sb, in_=sel_d[:, :])

            wqt_sb = consts.tile([P, 2, D], F16)
            wkt_sb = consts.tile([P, 2, D], F16)
            wvt_sb = consts.tile([P, 2, D], F16)
            for w_sb, w_d in ((wqt_sb, wqt_d), (wkt_sb, wkt_d), (wvt_sb, wvt_d)):
                for kc in range(2):
                    nc.sync.dma_start(
                        out=w_sb[:, kc, :],
                        in_=w_d[kc * P : (kc + 1) * P, :],
                    )

            # Residents (fp16): x.T and Q.T, [128, 2, 8192] each (4 MB).
            xt_res = consts.tile([P, 2, T * N], F16)
            qt_res = consts.tile([P, 2, T * N], BF16)

            # Softmax stats in PSUM (accumulate across chunks):
            # rows 0:64 sum_e, rows 64:128 sum_xe (concurrent col bands).
            pstat = ps.tile([P, 1024], F32, tag="b2", bufs=3, name="pstat")

            # ---------------- Phase A: stream x; stats; x.T; Q.T
            # PE queue is in-order: transposes(c) depend only on the
            # (prefetched) x chunk, while stats(c)/Q(c) wait on ACT/DVE
            # products.  Emitting stats/Q one chunk late keeps the PE fed.
            xa_t = {}
            e_t = {}

            def a_load(c):
                t0 = c * CHUNK_T
                xa = xa_pool.tile([P, 4, D], F16, name="xa")
                for ti in range(CHUNK_T):
                    # Pool SWDGE converts fp32 -> fp16 during the load.
                    nc.gpsimd.dma_start(
                        out=xa[ti * 64 : (ti + 1) * 64],
                        in_=x_d[t0 + ti].rearrange("(s p) d -> p s d", p=64),
                    )
                xa_t[c] = xa

            def a_transposes(c):
                xa = xa_t[c]
                for dc in range(2):
                    pt = ps.tile([P, CHUNK_TN], F16, tag="b1", bufs=2, name="pt")
                    for s in range(4):
                        nc.tensor.transpose(
                            pt[:, s * P : (s + 1) * P],
                            xa[:, s, dc * P : (dc + 1) * P],
                            eye16,
                        )
                    nc.vector.tensor_copy(
                        xt_res[:, dc, c * CHUNK_TN : (c + 1) * CHUNK_TN], pt
                    )

            def a_exp(c):
                xa = xa_t[c]
                e2 = e_pool.tile([P, 4 * D], F16, tag="e2", name="e2")
                nc.scalar.activation(e2, xa.rearrange("p s d -> p (s d)"), AF.Exp)
                xe2 = e_pool.tile([P, 4 * D], F16, tag="xe2", name="xe2")
                nc.vector.tensor_mul(xe2, xa.rearrange("p s d -> p (s d)"), e2)
                e_t[c] = (e2, xe2)

            def a_stats(c):
                e2, xe2 = e_t.pop(c)
                for half in range(2):
                    nc.tensor.matmul(
                        pstat[0:64, half * 512 : (half + 1) * 512],
                        sel_sb,
                        e2[:, half * 512 : (half + 1) * 512],
                        start=(c == 0),
                        stop=(c == NCHUNKS - 1),
                        tile_position=(0, 0),
                        skip_group_check=True,
                    )
                    nc.tensor.matmul(
                        pstat[64:128, half * 512 : (half + 1) * 512],
                        sel_sb,
                        xe2[:, half * 512 : (half + 1) * 512],
                        start=(c == 0),
                        stop=(c == NCHUNKS - 1),
                        tile_position=(0, 64),
                        skip_group_check=True,
                    )

            def a_q(c):
                xa_t.pop(c, None)
                for jc in range(2):
                    pq = ps.tile([P, CHUNK_TN], F32, tag="b1", bufs=2, name="pq")
                    for kc in range(2):
                        nc.tensor.matmul(
                            pq,
                            wqt_sb[:, kc, jc * P : (jc + 1) * P],
                            xt_res[:, kc, c * CHUNK_TN : (c + 1) * CHUNK_TN],
                            start=(kc == 0),
                            stop=(kc == 1),
                        )
                    dst = qt_res[:, jc, c * CHUNK_TN : (c + 1) * CHUNK_TN]
                    if jc == 0:
                        nc.scalar.activation(dst, pq, AF.Copy)
                    else:
                        nc.vector.tensor_copy(dst, pq)

            a_load(0)
            a_load(1)
            for c in range(NCHUNKS):
                if c + 2 < NCHUNKS:
                    a_load(c + 2)
                a_exp(c)
                a_transposes(c)
                if c >= 1:
                    a_stats(c - 1)
                    a_q(c - 1)
            a_stats(NCHUNKS - 1)
            a_q(NCHUNKS - 1)

            # ---------------- Phase B: TAtt, K, V, vt2
            se_sb = misc.tile([64, 1024], F32)
            nc.scalar.activation(se_sb, pstat[0:64, :], AF.Copy)
            sxe_sb = misc.tile([64, 1024], F32)
            nc.vector.tensor_copy(sxe_sb, pstat[64:128, :])
            rec = misc.tile([64, 1024], F32)
            nc.vector.reciprocal_approx_fast(out=rec, in_=se_sb)
            tatt_nat = misc.tile([64, 1024], F32)  # [p64, (s4, d256)]
            nc.vector.tensor_mul(tatt_nat, sxe_sb, rec)

            # TAtt.T [d, n] fp16 (n = s*64+p).
            tatt_t = consts.tile([P, 2, N], F16)
            for dc in range(2):
                ptb = ps.tile([P, N], F32, tag="b1", bufs=2, name="ptb")
                for s in range(4):
                    nc.tensor.transpose(
                        ptb[:, s * 64 : (s + 1) * 64],
                        tatt_nat[:, s * 256 + dc * P : s * 256 + (dc + 1) * P],
                        eye[0:64, 0:64],
                    )
                nc.vector.tensor_copy(tatt_t[:, dc, :], ptb)

            kt_sb = consts.tile([P, 2, N], BF16)  # K.T [j, m], pre-scaled
            for jc in range(2):
                pk = ps.tile([P, N], F32, tag="b1", bufs=2, name="pk")
                for kc in range(2):
                    nc.tensor.matmul(
                        pk,
                        wkt_sb[:, kc, jc * P : (jc + 1) * P],
                        tatt_t[:, kc, :],
                        start=(kc == 0),
                        stop=(kc == 1),
                    )
                nc.scalar.activation(kt_sb[:, jc, :], pk, AF.Copy)

            v_sb = consts.tile([P, 2, D], BF16)  # V [m, j]
            for mc in range(2):
                pv = ps.tile([P, D], F32, tag="b1", bufs=2, name="pv")
                for kc in range(2):
                    nc.tensor.matmul(
                        pv,
                        tatt_t[:, kc, mc * P : (mc + 1) * P],
                        wvt_sb[:, kc, :],
                        start=(kc == 0),
                        stop=(kc == 1),
                    )
                nc.vector.tensor_copy(v_sb[:, mc, :], pv)

            vt_sb = misc.tile([P, 2, N], F32)  # V.T [j, n]
            for jc in range(2):
                pt2 = ps.tile([P, N], BF16, tag="b1", bufs=2, name="pt2")
                for mc in range(2):
                    nc.tensor.transpose(
                        pt2[:, mc * P : (mc + 1) * P],
                        v_sb[:, mc, jc * P : (jc + 1) * P],
                        eye_b,
                    )
                nc.vector.tensor_copy(vt_sb[:, jc, :], pt2)

            # vt2: V.T in chunk column order (col = s*128 + t*64 + p).
            vt2 = consts.tile([P, 2, CHUNK_TN], F32)
            for hg in range(2):
                for s in range(4):
                    for ti in range(CHUNK_T):
                        nc.gpsimd.tensor_copy(
                            vt2[:, hg, s * P + ti * 64 : s * P + ti * 64 + 64],
                            vt_sb[:, hg, s * 64 : (s + 1) * 64],
                        )

            # ---------------- Phase C: S -> relu; A@V/out one chunk behind
            a_store = {}

            def s_phase_hg(c, hg):
                nrelu = 4 * hg
                if True:
                    for mc in range(2):
                        for rp in range(2):
                            ps2 = ps.tile(
                                [P, 2 * CHUNK_TN],
                                F32,
                                tag="b2",
                                bufs=3,
                                name=f"ps{hg}{mc}{rp}",
                            )
                            for rh in range(2):
                                r = rp * 2 + rh
                                nc.tensor.matmul(
                                    ps2[:, rh * CHUNK_TN : (rh + 1) * CHUNK_TN],
                                    kt_sb[
                                        r * 32 : (r + 1) * 32,
                                        hg,
                                        mc * P : (mc + 1) * P,
                                    ],
                                    qt_res[
                                        r * 32 : (r + 1) * 32,
                                        hg,
                                        c * CHUNK_TN : (c + 1) * CHUNK_TN,
                                    ],
                                    start=True,
                                    stop=True,
                                    tile_position=(r * 32, 0),
                                )
                            a2 = a_pool.tile(
                                [P, 2 * CHUNK_TN],
                                BF16,
                                tag="at",
                                name=f"a{hg}{mc}{rp}",
                            )
                            # Both engines drain each tile in parallel so
                            # the PSUM slot frees ~2x sooner for the next
                            # S quad (split sized to balance ACT vs DVE).
                            nc.scalar.activation(
                                a2[:, 0:544], ps2[:, 0:544], AF.Relu
                            )
                            nc.vector.tensor_scalar_max(
                                a2[:, 544:1024], ps2[:, 544:1024], 0.0
                            )
                            nrelu += 1
                            for rh in range(2):
                                a_store[(c, hg, rp * 2 + rh, mc)] = a2[
                                    :, rh * CHUNK_TN : (rh + 1) * CHUNK_TN
                                ]

            o_sb_store = {}

            def av_phase_hg(c, hg):
                po = ps.tile(
                    [P, CHUNK_TN], F32, tag="b1", bufs=2, name=f"po{hg}"
                )
                for mc in range(2):
                    for r in range(4):
                        h = hg * 4 + r
                        nc.tensor.matmul(
                            po[r * 32 : (r + 1) * 32, :],
                            v_sb[:, mc, h * 32 : (h + 1) * 32],
                            a_store.pop((c, hg, r, mc)),
                            start=(mc == 0),
                            stop=(mc == 1),
                            tile_position=(0, r * 32),
                            skip_group_check=True,
                        )
                o_sb = o_pool.tile(
                    [P, CHUNK_TN], F32, tag=f"ob{hg}", name="o_sb"
                )
                nc.vector.scalar_tensor_tensor(
                    out=o_sb,
                    in0=po,
                    scalar=1.0,
                    in1=vt2[:, hg, :],
                    op0=mybir.AluOpType.mult,
                    op1=mybir.AluOpType.add,
                )
                o_sb_store[(c, hg)] = o_sb

            def out_tail(c):
                t0 = c * CHUNK_T
                o_sbs = [o_sb_store.pop((c, 0)), o_sb_store.pop((c, 1))]
                # PE transpose -> [(t, p), (s, hg, j)]; contiguous out rows.
                po2 = ps.tile(
                    [P, 2 * CHUNK_TN], F32, tag="b2", bufs=3, name="po2"
                )
                for hg in range(2):
                    for s in range(4):
                        nc.tensor.transpose(
                            po2[:, s * 256 + hg * P : s * 256 + (hg + 1) * P],
                            o_sbs[hg][:, s * P : (s + 1) * P],
                            eye,
                        )
                o2 = o_pool.tile([P, 2 * CHUNK_TN], F32, tag="o2", name="o2")
                nc.scalar.activation(o2, po2, AF.Copy)
                for ti in range(CHUNK_T):
                    nc.sync.dma_start(
                        out=out_d[t0 + ti].rearrange("(s p) d -> p s d", p=64),
                        in_=o2[ti * 64 : (ti + 1) * 64].rearrange(
                            "p (s d) -> p s d", s=4
                        ),
                    )

            for c in range(NCHUNKS):
                s_phase_hg(c, 0)
                s_phase_hg(c, 1)
                if c >= 1:
                    av_phase_hg(c - 1, 0)
                    av_phase_hg(c - 1, 1)
                    out_tail(c - 1)
            av_phase_hg(NCHUNKS - 1, 0)
            av_phase_hg(NCHUNKS - 1, 1)
            out_tail(NCHUNKS - 1)

    nc.finalize()
    return nc


def _in_maps(inputs) -> list:
    x = np.ascontiguousarray(np.asarray(inputs["x"], dtype=np.float32))
    w_q = np.asarray(inputs["W_Q"], dtype=np.float32)
    w_k = np.asarray(inputs["W_K"], dtype=np.float32)
    w_v = np.asarray(inputs["W_V"], dtype=np.float32)

    wqt = np.ascontiguousarray(w_q.T).astype(np.float16)
    wkt = (np.ascontiguousarray(w_k.T) * np.float32(1.0 / np.sqrt(DH))).astype(
        np.float16
    )
    wvt = np.ascontiguousarray(w_v.T).astype(np.float16)
    sel = np.zeros((P, 64), dtype=np.float16)
    for t in range(CHUNK_T):
        sel[t * 64 : (t + 1) * 64] = np.eye(64, dtype=np.float16)

    return [
        {
            "x": np.ascontiguousarray(x[b]),
            "wqt": wqt,
            "wkt": wkt,
            "wvt": wvt,
            "sel": sel,
        }
        for b in range(B)
    ]


def kernel(**inputs) -> np.ndarray:
    if "nc" not in _CACHE:
        _CACHE["nc"] = _build_program()
    nc = _CACHE["nc"]
    in_maps = _in_maps(inputs)
    res = run_bass_kernel_spmd(nc, in_maps, core_ids=list(range(B)))
    out = np.stack([res.results[b]["out"] for b in range(B)], axis=0)
    return out.reshape(B, T, N, D)
